# revision 12
# baseline (speedup 1.0000x reference)
"""Trainium2 Bass kernel for nn_Attn2Quad (squared-attention with mask).

reference math (per b,h):
    x   = q @ k^T * SCALE + (1-mask)*NEG + C        # [T, T]
    num = x * x
    den = num.sum(-1) + EPS                          # [T]
    p   = num / den                                  # [T, T]  (output)
    out = p @ v                                      # [T, D]  (output)

Sharding: B*H = 48 (b,h) pairs split across 8 cores, 6 per core (each core
covers a single b, so one mask row per core).

Per-core kernel strategy (T=2048, D=64, all matmuls fp32r):
  - Build QT' = [SCALE*q^T ; ones] and KT' = [k^T ; maskc] as [65, T] SBUF
    tiles (PE transposes of natural [128,64] tiles).  The 65th contraction
    row makes the PE matmul emit x (mask included) directly.
  - Phase A (per 128-wide k-block, two 1024-wide q-hemispheres):
    xT = KT'_blk^T @ QT' -> PSUM [128k, 1024]; numT = Square(xT) -> SBUF
    fp32r; accumulate outT' += V'_blk^T @ numT where V' = [v | ones] so
    partition 64 of outT' [65, 1024] accumulates den per q.
  - Interphase (per hemisphere): PE-transpose outT' -> [q, 65] blocks;
    invden = 1/(den+EPS), r = sqrt(invden); out = outT * invden -> DMA.
  - Phase B (per 128-wide q-block): x = QT'_blk^T @ KT' -> PSUM [128q, T];
    p = Square(x * r[q]) -> SBUF -> DMA (row-contiguous 1 MB stores).
  - Software pipeline: phase B of head i is emitted interleaved with phase A
    of head i+1 so all engines + DMA stay busy; square work is split across
    ScalarE / VectorE / (VectorE+GpSimd) to balance engine occupancy.
"""

import numpy as np

B, H, T, D = 4, 12, 2048, 64
C = 3.0
EPS = 1e-6
SCALE = 0.125  # 1/sqrt(64)
NEG = -10000.0
N_CORES = 8
BH_PER_CORE = (B * H) // N_CORES  # 6
NT = T // 128  # 16

_BUILD_CACHE = {}


def build_bass(n_bh=BH_PER_CORE):
    """Build the per-core Bass program (cached)."""
    if n_bh in _BUILD_CACHE:
        return _BUILD_CACHE[n_bh]

    import concourse.bacc as bacc
    import concourse.mybir as mybir
    import concourse.tile as tile
    from concourse.masks import make_identity

    F32 = mybir.dt.float32
    F32R = mybir.dt.float32r
    I32 = mybir.dt.int32
    AF = mybir.ActivationFunctionType
    MULT = mybir.AluOpType.mult

    nc = bacc.Bacc("TRN2", target_bir_lowering=False)
    q_in = nc.dram_tensor("q", (n_bh, T, D), F32, kind="ExternalInput")
    k_in = nc.dram_tensor("k", (n_bh, T, D), F32, kind="ExternalInput")
    v_in = nc.dram_tensor("v", (n_bh, T, D), F32, kind="ExternalInput")
    m_in = nc.dram_tensor("mask", (1, T), I32, kind="ExternalInput")
    p_out = nc.dram_tensor("p", (n_bh, T, T), F32, kind="ExternalOutput")
    o_out = nc.dram_tensor("out", (n_bh, T, D), F32, kind="ExternalOutput")

    with tile.TileContext(nc) as tc:
        with tc.tile_pool(name="const", bufs=1) as const, \
             tc.tile_pool(name="io", bufs=2) as io, \
             tc.tile_pool(name="work", bufs=2) as work, \
             tc.tile_pool(name="numt", bufs=3) as numt_pool, \
             tc.tile_pool(name="pp", bufs=3) as pp, \
             tc.tile_pool(name="ps", bufs=1, space="PSUM") as ps:

            ident = const.tile([128, 128], F32)
            make_identity(nc, ident)

            m_sb = const.tile([1, T], I32)
            nc.sync.dma_start(m_sb, m_in.ap())
            # maskc = mask*(-NEG) + (NEG + C):  1 -> C,  0 -> NEG + C
            maskc = const.tile([1, T], F32R)
            nc.vector.tensor_scalar(maskc, m_sb, -NEG, NEG + C,
                                    MULT, mybir.AluOpType.add)

            def ps1024(shape):
                return ps.tile(shape, F32, tag="ps1024", bufs=3, name="pst")

            def emit_loads(bh):
                st = {"bh": bh}
                nat_q = io.tile([128, NT, D], F32, tag="natq", name="natq")
                nat_k = io.tile([128, NT, D], F32, tag="natk", name="natk")
                nat_v = io.tile([128, NT, D], F32, tag="natv", name="natv")
                nc.sync.dma_start(
                    nat_q, q_in.ap()[bh].rearrange("(o p) f -> p o f", p=128))
                nc.sync.dma_start(
                    nat_k, k_in.ap()[bh].rearrange("(o p) f -> p o f", p=128))
                nc.sync.dma_start(
                    nat_v, v_in.ap()[bh].rearrange("(o p) f -> p o f", p=128))

                qt = work.tile([65, T], F32R, tag="qt", name="qt")
                kt = work.tile([65, T], F32R, tag="kt", name="kt")
                vp = work.tile([128, NT, 65], F32R, tag="vp", name="vp")

                # V' = [v | ones]
                nc.gpsimd.memset(vp.bitcast(mybir.dt.uint32), 0x3F800000)
                nc.vector.tensor_copy(vp[:, :, 0:64], nat_v)

                # QT'/KT' via PE transposes (1024-wide chunks through PSUM)
                for half in range(2):
                    cols = slice(half * 1024, (half + 1) * 1024)
                    tq = ps1024([64, 1024])
                    for j in range(8):
                        o = half * 8 + j
                        nc.tensor.transpose(
                            tq[:, j * 128:(j + 1) * 128], nat_q[:, o, :], ident)
                    nc.scalar.mul(qt[0:64, cols], tq, SCALE)
                    tk = ps1024([64, 1024])
                    for j in range(8):
                        o = half * 8 + j
                        nc.tensor.transpose(
                            tk[:, j * 128:(j + 1) * 128], nat_k[:, o, :], ident)
                    nc.vector.tensor_copy(kt[0:64, cols], tk)
                nc.gpsimd.memset(qt[64:65, :].bitcast(mybir.dt.uint32), 0x3F800000)
                nc.vector.tensor_copy(kt[64:65, :], maskc)

                st["qt"], st["kt"], st["vp"] = qt, kt, vp
                st["r"] = work.tile([128, NT], F32, tag="r", name="r")
                st["den"] = work.tile([128, NT], F32, tag="den", name="den")
                st["iv"] = work.tile([128, NT], F32, tag="iv", name="iv")
                st["out_sb"] = work.tile([128, NT, D], F32, tag="outsb",
                                         name="outsb")
                return st

            def emit_phaseA_chunk(st, i):
                # i in [0, 32): h2 = i // 16 (q hemisphere), kb = i % 16
                h2, kb = divmod(i, NT)
                qt, kt, vp = st["qt"], st["kt"], st["vp"]
                c0 = h2 * 1024
                if kb == 0:
                    st["outT_ps"] = ps.tile([65, 1024], F32, tag="outT",
                                            name="outT")
                outT_ps = st["outT_ps"]
                ksl = slice(kb * 128, (kb + 1) * 128)
                xT = ps1024([128, 1024])
                nc.tensor.matmul(xT[:, 0:512], kt[:, ksl],
                                 qt[:, c0:c0 + 512], start=True, stop=True)
                nc.tensor.matmul(xT[:, 512:1024], kt[:, ksl],
                                 qt[:, c0 + 512:c0 + 1024],
                                 start=True, stop=True)
                numT = numt_pool.tile([128, 1024], F32R, tag="numT",
                                      name="numT")
                # phase-A square split: mostly ScalarE, some VectorE
                if i % 16 in (5, 10, 15):
                    nc.vector.tensor_copy(numT, xT)
                    nc.vector.tensor_tensor(numT, numT, numT, MULT)
                else:
                    nc.scalar.activation(numT, xT, AF.Square)
                nc.tensor.matmul(outT_ps[:, 0:512], vp[:, kb, :],
                                 numT[:, 0:512],
                                 start=(kb == 0), stop=(kb == NT - 1))
                nc.tensor.matmul(outT_ps[:, 512:1024], vp[:, kb, :],
                                 numT[:, 512:1024],
                                 start=(kb == 0), stop=(kb == NT - 1))

            def emit_interphase_half(st, h2):
                bh = st["bh"]
                r, den, iv, out_sb = st["r"], st["den"], st["iv"], st["out_sb"]
                outT_ps = st["outT_ps"]
                outT_sb = work.tile([65, 1024], F32, tag="outTsb",
                                    name="outTsb")
                nc.scalar.copy(outT_sb, outT_ps)
                osl = slice(h2 * 8, (h2 + 1) * 8)
                op_ps = ps1024([128, 8, 128])
                for j in range(8):
                    nc.tensor.transpose(
                        op_ps[:, j, 0:65],
                        outT_sb[:, j * 128:(j + 1) * 128],
                        ident[0:65, 0:65])
                nc.vector.tensor_scalar_add(den[:, osl], op_ps[:, :, 64], EPS)
                nc.vector.reciprocal(iv[:, osl], den[:, osl])
                nc.scalar.activation(r[:, osl], iv[:, osl], AF.Sqrt)
                nc.vector.tensor_tensor(
                    out_sb[:, osl, :], op_ps[:, :, 0:64],
                    iv[:, osl][:, :, None].to_broadcast((128, 8, D)), MULT)
                if h2 == 1:
                    nc.sync.dma_start(
                        o_out.ap()[bh].rearrange("(o p) f -> p o f", p=128),
                        out_sb)

            def emit_phaseB_chunk(st, i):
                # i in [0, 32): qb = i // 2, hf = i % 2
                qb, hf = divmod(i, 2)
                bh = st["bh"]
                qt, kt, r = st["qt"], st["kt"], st["r"]
                qsl = slice(qb * 128, (qb + 1) * 128)
                if hf == 0:
                    st["p_sb"] = pp.tile([128, T], F32, tag="p", name="psb")
                p_sb = st["p_sb"]
                c0 = hf * 1024
                xB = ps1024([128, 1024])
                nc.tensor.matmul(xB[:, 0:512], qt[:, qsl],
                                 kt[:, c0:c0 + 512], start=True, stop=True)
                nc.tensor.matmul(xB[:, 512:1024], qt[:, qsl],
                                 kt[:, c0 + 512:c0 + 1024],
                                 start=True, stop=True)
                dst = p_sb[:, c0:c0 + 1024]
                # phase-B split: 5/8 VectorE-mul + GpSimd-square, 3/8 ScalarE
                if i % 8 < 5:
                    nc.vector.tensor_scalar_mul(dst, xB, r[:, qb:qb + 1])
                    nc.gpsimd.tensor_tensor(dst, dst, dst, MULT)
                else:
                    nc.scalar.activation(dst, xB, AF.Square, bias=0.0,
                                         scale=r[:, qb:qb + 1])
                if hf == 1:
                    nc.sync.dma_start(p_out.ap()[bh, qsl, :], p_sb)

            # B-chunks become eligible one hemisphere (8 q-blocks) after the
            # matching interphase; pump them 1:1 against A-chunks so the
            # pipeline fill/drain is only half a head on each end.
            pending_b = []

            def pump_b(force=False):
                # keep an >=8-chunk backlog so the interphase r-latency is
                # always hidden behind other work when a B-chunk is emitted
                if pending_b and (force or len(pending_b) > 8):
                    st, i = pending_b.pop(0)
                    emit_phaseB_chunk(st, i)

            for bh in range(n_bh):
                cur = emit_loads(bh)
                for i in range(32):
                    emit_phaseA_chunk(cur, i)
                    pump_b()
                    if i == 15:
                        emit_interphase_half(cur, 0)
                        pending_b.extend((cur, j) for j in range(16))
                    elif i == 31:
                        emit_interphase_half(cur, 1)
                        pending_b.extend((cur, j) for j in range(16, 32))
            while pending_b:
                pump_b(force=True)

    nc.finalize()
    _BUILD_CACHE[n_bh] = nc
    return nc


def kernel(q, k, v, attention_mask):
    """Full-problem entry: shard across 8 cores, run, reassemble."""
    from concourse import bass_utils

    q = np.ascontiguousarray(q, dtype=np.float32)
    k = np.ascontiguousarray(k, dtype=np.float32)
    v = np.ascontiguousarray(v, dtype=np.float32)
    attention_mask = np.ascontiguousarray(attention_mask, dtype=np.int32)

    qf = q.reshape(B * H, T, D)
    kf = k.reshape(B * H, T, D)
    vf = v.reshape(B * H, T, D)

    in_maps = []
    for c in range(N_CORES):
        g0 = c * BH_PER_CORE
        b = g0 // H
        in_maps.append({
            "q": qf[g0:g0 + BH_PER_CORE],
            "k": kf[g0:g0 + BH_PER_CORE],
            "v": vf[g0:g0 + BH_PER_CORE],
            "mask": attention_mask[b:b + 1],
        })

    nc = build_bass(BH_PER_CORE)
    res = bass_utils.run_bass_kernel_spmd(nc, in_maps,
                                          core_ids=list(range(N_CORES)))

    p_full = np.empty((B, H, T, T), dtype=np.float32)
    out_full = np.empty((B, H, T, D), dtype=np.float32)
    for c in range(N_CORES):
        g0 = c * BH_PER_CORE
        b = g0 // H
        h0 = g0 % H
        p_full[b, h0:h0 + BH_PER_CORE] = res.results[c]["p"]
        out_full[b, h0:h0 + BH_PER_CORE] = res.results[c]["out"]
    return (out_full, p_full)


# revision 24
# speedup vs baseline: 1.0315x; 1.0315x over previous
"""Trainium2 Bass kernel for nn_Attn2Quad (squared-attention with mask).

reference math (per b,h):
    x   = q @ k^T * SCALE + (1-mask)*NEG + C        # [T, T]
    num = x * x
    den = num.sum(-1) + EPS                          # [T]
    p   = num / den                                  # [T, T]  (output)
    out = p @ v                                      # [T, D]  (output)

Sharding: B*H = 48 (b,h) pairs split across 8 cores, 6 per core (each core
covers a single b, so one mask row per core).

Per-core kernel strategy (T=2048, D=64, all matmuls fp32r):
  - Build QT' = [SCALE*q^T ; ones] and KT' = [k^T ; maskc] as [65, T] SBUF
    tiles (PE transposes of natural [128,64] tiles).  The 65th contraction
    row makes the PE matmul emit x (mask included) directly.
  - Phase A (per 128-wide k-block, two 1024-wide q-hemispheres):
    xT = KT'_blk^T @ QT' -> PSUM [128k, 1024]; numT = Square(xT) -> SBUF
    fp32r; accumulate outT' += V'_blk^T @ numT where V' = [v | ones] so
    partition 64 of outT' [65, 1024] accumulates den per q.
  - Interphase (per hemisphere): PE-transpose outT' -> [q, 65] blocks;
    invden = 1/(den+EPS), r = sqrt(invden); out = outT * invden -> DMA.
  - Phase B (per 128-wide q-block): x = QT'_blk^T @ KT' -> PSUM [128q, T];
    p = Square(x * r[q]) -> SBUF -> DMA (row-contiguous 1 MB stores).
  - Software pipeline: phase B of head i is emitted interleaved with phase A
    of head i+1 so all engines + DMA stay busy; square work is split across
    ScalarE / VectorE / (VectorE+GpSimd) to balance engine occupancy.
"""

import numpy as np

B, H, T, D = 4, 12, 2048, 64
C = 3.0
EPS = 1e-6
SCALE = 0.125  # 1/sqrt(64)
NEG = -10000.0
N_CORES = 8
BH_PER_CORE = (B * H) // N_CORES  # 6
NT = T // 128  # 16

_BUILD_CACHE = {}


def build_bass(n_bh=BH_PER_CORE, reps=1, timing=False):
    """Build the per-core Bass program (cached).

    timing=True replaces the big p output with an internal DRAM scratch
    (same on-device DMA traffic, no host download) for exec-time probes.
    """
    key = (n_bh, reps, timing)
    if key in _BUILD_CACHE:
        return _BUILD_CACHE[key]

    import concourse.bacc as bacc
    import concourse.mybir as mybir
    import concourse.tile as tile
    from concourse.masks import make_identity

    F32 = mybir.dt.float32
    F32R = mybir.dt.float32r
    BF16 = mybir.dt.bfloat16
    I32 = mybir.dt.int32
    AF = mybir.ActivationFunctionType
    MULT = mybir.AluOpType.mult

    nc = bacc.Bacc("TRN2", target_bir_lowering=False)
    q_in = nc.dram_tensor("q", (n_bh, T, D), F32, kind="ExternalInput")
    k_in = nc.dram_tensor("k", (n_bh, T, D), F32, kind="ExternalInput")
    v_in = nc.dram_tensor("v", (n_bh, T, D), F32, kind="ExternalInput")
    m_in = nc.dram_tensor("mask", (1, T), I32, kind="ExternalInput")
    p_out = nc.dram_tensor("p", (n_bh, T, T), F32,
                           kind="Internal" if timing else "ExternalOutput")
    o_out = nc.dram_tensor("out", (n_bh, T, D), F32, kind="ExternalOutput")

    with tile.TileContext(nc) as tc:
        with tc.tile_pool(name="const", bufs=1) as const, \
             tc.tile_pool(name="io", bufs=2) as io, \
             tc.tile_pool(name="work", bufs=2) as work, \
             tc.tile_pool(name="numt", bufs=3) as numt_pool, \
             tc.tile_pool(name="pp", bufs=3) as pp, \
             tc.tile_pool(name="ps", bufs=1, space="PSUM") as ps:

            ident = const.tile([128, 128], F32)
            make_identity(nc, ident)

            m_sb = const.tile([1, T], I32)
            nc.sync.dma_start(m_sb, m_in.ap())
            # maskc = mask*(-NEG) + (NEG + C):  1 -> C,  0 -> NEG + C
            maskc = const.tile([1, T], BF16)
            nc.vector.tensor_scalar(maskc, m_sb, -NEG, NEG + C,
                                    MULT, mybir.AluOpType.add)

            def ps1024(shape):
                return ps.tile(shape, F32, tag="ps1024", bufs=3, name="pst")

            def emit_loads(bh):
                st = {"bh": bh}
                # q/k load with row-interleaved layout: partition p holds
                # rows {p*16+o}, a contiguous 4KB HBM run per partition
                # (full DMA line rate); the PSUM->SBUF copy after the PE
                # transposes un-permutes the columns for free via a strided
                # access pattern.  v must match KT' column order, so it keeps
                # the 256B-line layout.
                nat_q = io.tile([128, NT, D], F32, tag="natq", name="natq")
                nat_k = io.tile([128, NT, D], F32, tag="natk", name="natk")
                nat_v = io.tile([128, NT, D], F32, tag="natv", name="natv")
                nc.sync.dma_start(
                    nat_q, q_in.ap()[bh].rearrange("(p o) f -> p o f", o=NT))
                nc.sync.dma_start(
                    nat_k, k_in.ap()[bh].rearrange("(p o) f -> p o f", o=NT))
                nc.sync.dma_start(
                    nat_v, v_in.ap()[bh].rearrange("(o p) f -> p o f", p=128))

                qt = work.tile([65, T], BF16, tag="qt", name="qt", bufs=3)
                kt = work.tile([65, T], BF16, tag="kt", name="kt", bufs=3)
                vp = work.tile([128, NT, 65], F32R, tag="vp", name="vp", bufs=3)

                # V' = [v | ones]
                nc.gpsimd.memset(vp.bitcast(mybir.dt.uint32), 0x3F800000)
                nc.vector.tensor_copy(vp[:, :, 0:64], nat_v)

                # QT'/KT' via PE transposes (1024-wide chunks through PSUM).
                # nat tile block o holds rows {p*16+o}, so the transposed
                # block's column c is global row 16c+(8*half+j); the copy
                # un-permutes via a strided dest AP at no extra op cost.
                for half in range(2):
                    tq = ps1024([64, 8, 128])
                    for j in range(8):
                        nc.tensor.transpose(
                            tq[:, j, :], nat_q[:, half * 8 + j, :], ident)
                    qt_dst = qt[0:64, :].rearrange(
                        "p (c x) -> p c x", x=NT)[:, :, half * 8:half * 8 + 8]
                    nc.scalar.mul(qt_dst, tq.rearrange("p a b -> p b a"), SCALE)
                    tk = ps1024([64, 8, 128])
                    for j in range(8):
                        nc.tensor.transpose(
                            tk[:, j, :], nat_k[:, half * 8 + j, :], ident)
                    kt_dst = kt[0:64, :].rearrange(
                        "p (c x) -> p c x", x=NT)[:, :, half * 8:half * 8 + 8]
                    nc.vector.tensor_copy(kt_dst, tk.rearrange("p a b -> p b a"))
                nc.gpsimd.memset(qt[64:65, :].bitcast(mybir.dt.uint16), 0x3F80)
                nc.vector.tensor_copy(kt[64:65, :], maskc)

                st["qt"], st["kt"], st["vp"] = qt, kt, vp
                st["r"] = work.tile([128, NT], F32, tag="r", name="r")
                st["den"] = work.tile([128, NT], F32, tag="den", name="den")
                st["iv"] = work.tile([128, NT], F32, tag="iv", name="iv")
                st["out_sb"] = work.tile([128, NT, D], F32, tag="outsb",
                                         name="outsb")
                return st

            def emit_phaseA_chunk(st, i):
                # i in [0, 32): h2 = i // 16 (q hemisphere), kb = i % 16
                h2, kb = divmod(i, NT)
                qt, kt, vp = st["qt"], st["kt"], st["vp"]
                c0 = h2 * 1024
                if kb == 0:
                    st["outT_ps"] = ps.tile([65, 1024], F32, tag="outT",
                                            name="outT")
                outT_ps = st["outT_ps"]
                ksl = slice(kb * 128, (kb + 1) * 128)
                xT = ps1024([128, 1024])
                nc.tensor.matmul(xT[:, 0:512], kt[:, ksl],
                                 qt[:, c0:c0 + 512], start=True, stop=True)
                nc.tensor.matmul(xT[:, 512:1024], kt[:, ksl],
                                 qt[:, c0 + 512:c0 + 1024],
                                 start=True, stop=True)
                numT = numt_pool.tile([128, 1024], F32R, tag="numT",
                                      name="numT", bufs=4)
                # phase-A square split: mostly ScalarE, some VectorE
                if i % 16 in (5, 10, 15):
                    nc.vector.tensor_copy(numT, xT)
                    nc.vector.tensor_tensor(numT, numT, numT, MULT)
                else:
                    nc.scalar.activation(numT, xT, AF.Square)
                nc.tensor.matmul(outT_ps[:, 0:512], vp[:, kb, :],
                                 numT[:, 0:512],
                                 start=(kb == 0), stop=(kb == NT - 1))
                nc.tensor.matmul(outT_ps[:, 512:1024], vp[:, kb, :],
                                 numT[:, 512:1024],
                                 start=(kb == 0), stop=(kb == NT - 1))

            def emit_interphase_half(st, h2):
                bh = st["bh"]
                r, den, iv, out_sb = st["r"], st["den"], st["iv"], st["out_sb"]
                outT_ps = st["outT_ps"]
                outT_sb = work.tile([65, 1024], F32, tag="outTsb",
                                    name="outTsb")
                nc.scalar.copy(outT_sb, outT_ps)
                osl = slice(h2 * 8, (h2 + 1) * 8)
                op_ps = ps1024([128, 8, 128])
                for j in range(8):
                    nc.tensor.transpose(
                        op_ps[:, j, 0:65],
                        outT_sb[:, j * 128:(j + 1) * 128],
                        ident[0:65, 0:65])
                nc.vector.tensor_scalar_add(den[:, osl], op_ps[:, :, 64], EPS)
                nc.vector.reciprocal(iv[:, osl], den[:, osl])
                nc.scalar.activation(r[:, osl], iv[:, osl], AF.Sqrt)
                nc.vector.tensor_tensor(
                    out_sb[:, osl, :], op_ps[:, :, 0:64],
                    iv[:, osl][:, :, None].to_broadcast((128, 8, D)), MULT)
                if h2 == 1:
                    nc.sync.dma_start(
                        o_out.ap()[bh].rearrange("(o p) f -> p o f", p=128),
                        out_sb)

            def emit_phaseB_chunk(st, i):
                # i in [0, 32): qb = i // 2, hf = i % 2
                qb, hf = divmod(i, 2)
                bh = st["bh"]
                qt, kt, r = st["qt"], st["kt"], st["r"]
                qsl = slice(qb * 128, (qb + 1) * 128)
                if hf == 0:
                    st["p_sb"] = pp.tile([128, T], F32, tag="p", name="psb", bufs=4)
                p_sb = st["p_sb"]
                c0 = hf * 1024
                xB = ps1024([128, 1024])
                nc.tensor.matmul(xB[:, 0:512], qt[:, qsl],
                                 kt[:, c0:c0 + 512], start=True, stop=True)
                nc.tensor.matmul(xB[:, 512:1024], qt[:, qsl],
                                 kt[:, c0 + 512:c0 + 1024],
                                 start=True, stop=True)
                dst = p_sb[:, c0:c0 + 1024]
                # phase-B split: 1/4 VectorE-mul + GpSimd-square,
                # 1/4 VectorE 2-op, 1/2 ScalarE single-op
                m = i % 4
                if m == 1:
                    nc.vector.tensor_scalar_mul(dst, xB, r[:, qb:qb + 1])
                    nc.gpsimd.tensor_tensor(dst, dst, dst, MULT)
                elif m == 3:
                    nc.vector.tensor_scalar_mul(dst, xB, r[:, qb:qb + 1])
                    nc.vector.tensor_tensor(dst, dst, dst, MULT)
                else:
                    nc.scalar.activation(dst, xB, AF.Square, bias=0.0,
                                         scale=r[:, qb:qb + 1])
                if hf == 1:
                    nc.sync.dma_start(p_out.ap()[bh, qsl, :], p_sb)

            # B-chunks become eligible one hemisphere (8 q-blocks) after the
            # matching interphase; pump them 1:1 against A-chunks so the
            # pipeline fill/drain is only half a head on each end.
            pending_b = []

            def pump_b():
                if pending_b:
                    st, i = pending_b.pop(0)
                    emit_phaseB_chunk(st, i)

            def emit_body():
                # Pump schedule keeps the popped B-chunk's r at least ~5
                # slots old (hides the interphase latency chain) while the
                # store queue never runs dry: half-rate pumping in the first
                # half of each A-sweep, 1.5x in the second half.
                nxt = emit_loads(0)
                for bh in range(n_bh):
                    cur, nxt = nxt, None
                    for i in range(32):
                        emit_phaseA_chunk(cur, i)
                        if i < 16:
                            if i % 2 == 0:
                                pump_b()
                        else:
                            pump_b()
                            if i % 2 == 1:
                                pump_b()
                        if i == 2 and bh + 1 < n_bh:
                            nxt = emit_loads(bh + 1)
                        elif i == 15:
                            emit_interphase_half(cur, 0)
                            pending_b.extend((cur, j) for j in range(16))
                        elif i == 31:
                            emit_interphase_half(cur, 1)
                            pending_b.extend((cur, j) for j in range(16, 32))
                while pending_b:
                    pump_b()

            if reps == 1:
                emit_body()
            else:
                with tc.For_i(0, reps, 1):
                    emit_body()

    nc.finalize()
    _BUILD_CACHE[key] = nc
    return nc


def kernel(q, k, v, attention_mask):
    """Full-problem entry: shard across 8 cores, run, reassemble."""
    from concourse import bass_utils

    q = np.ascontiguousarray(q, dtype=np.float32)
    k = np.ascontiguousarray(k, dtype=np.float32)
    v = np.ascontiguousarray(v, dtype=np.float32)
    attention_mask = np.ascontiguousarray(attention_mask, dtype=np.int32)

    qf = q.reshape(B * H, T, D)
    kf = k.reshape(B * H, T, D)
    vf = v.reshape(B * H, T, D)

    in_maps = []
    for c in range(N_CORES):
        g0 = c * BH_PER_CORE
        b = g0 // H
        in_maps.append({
            "q": qf[g0:g0 + BH_PER_CORE],
            "k": kf[g0:g0 + BH_PER_CORE],
            "v": vf[g0:g0 + BH_PER_CORE],
            "mask": attention_mask[b:b + 1],
        })

    nc = build_bass(BH_PER_CORE)
    res = bass_utils.run_bass_kernel_spmd(nc, in_maps,
                                          core_ids=list(range(N_CORES)))

    p_full = np.empty((B, H, T, T), dtype=np.float32)
    out_full = np.empty((B, H, T, D), dtype=np.float32)
    for c in range(N_CORES):
        g0 = c * BH_PER_CORE
        b = g0 // H
        h0 = g0 % H
        p_full[b, h0:h0 + BH_PER_CORE] = res.results[c]["p"]
        out_full[b, h0:h0 + BH_PER_CORE] = res.results[c]["out"]
    return (out_full, p_full)
